# revision 6
# baseline (speedup 1.0000x reference)
"""Trainium2 Bass kernel for the batched contrastive (NT-Xent-style) loss.

Problem (hardcoded shapes): z1, z2: [4, 256, 64, 64] f32.
  h = transpose(reshape(z, [4, 256, 4096]))        # [b, n=4096, c=256]
  a, b = l2-normalize rows of h1, h2
  semi(x, y): refl = exp(x@x^T/tau); between = exp(x@y^T/tau)
              loss_i = -log(between_ii / (refl_sum_i + between_sum_i - refl_ii))
  out = mean((semi(a,b) + semi(b,a))/2)

Per batch element the device needs only:
  sA_i = rowsum exp(a@a^T/tau), sB_i = rowsum exp(b@b^T/tau),
  sC_i = rowsum exp(a@b^T/tau), tC_j = colsum exp(a@b^T/tau),
  dots_i = a_i.b_i/tau
Then l1 = log(sA+sC-e^{1/tau}) - dots, l2 = log(sB+tC-e^{1/tau}) - dots.

Sharding: 8 cores = 4 batch elements x 2 row-halves. Each core receives a
concat input [z[b] | z[b][:, half]] of shape [256, 6144] per side so the
compiled SPMD program is identical across cores: lhsT tiles come from the
trailing 2048 columns (this core's output rows), rhs from the leading 4096.
Both operands are normalized on-device and pre-scaled by 1/sqrt(tau) so the
Gram matmul directly produces the exp() argument.
"""

import numpy as np

import concourse.bacc as bacc
import concourse.bass as bass  # noqa: F401  (MemorySpace etc.)
import concourse.mybir as mybir
import concourse.tile as tile
from concourse.bass_utils import run_bass_kernel_spmd

TAU = 0.4
P = 128          # partitions
C = 256          # channels (contraction dim) = 2 k-tiles
KT = 2
NF = 4096        # n (full columns)
NH = 2048        # rows per core
NCAT = NF + NH   # 6144
CH = 512         # matmul free-dim chunk
STRIPE = 2048    # psum stripe width (4 banks)
F32 = mybir.dt.float32
BF16 = mybir.dt.bfloat16

# out layout (fp32, 12288):
#  [0:2048)      sA, stored as [128, 16]: dram[p*16 + I] = sA_row(I*128+p)
#  [2048:4096)   sB, same layout
#  [4096:6144)   sC, same layout
#  [6144:10240)  csC partial colsums over this core's rows, natural j order
#  [10240:12288) dots (a_i.b_i)/tau for this core's rows, natural order
OUT_SIZE = 3 * NH + NF + NH

_PROGRAM = None


def _build_program():
    nc = bacc.Bacc(
        "TRN2",
        target_bir_lowering=False,
        debug=False,
        enable_asserts=False,
        num_devices=8,
    )
    zc1 = nc.dram_tensor("zc1", [C, NCAT], F32, kind="ExternalInput")
    zc2 = nc.dram_tensor("zc2", [C, NCAT], F32, kind="ExternalInput")
    out_t = nc.dram_tensor("out", [OUT_SIZE], F32, kind="ExternalOutput")

    Act = mybir.ActivationFunctionType

    with tile.TileContext(nc) as tc:
        with (
            tc.tile_pool(name="zstage", bufs=3) as zpool,
            tc.tile_pool(name="sqpool", bufs=2) as sqpool,
            tc.tile_pool(name="abpool", bufs=1) as abpool,
            tc.tile_pool(name="rnpool", bufs=3) as rnpool,
            tc.tile_pool(name="ecpool", bufs=2) as ecpool,
            tc.tile_pool(name="scrpool", bufs=2) as scrpool,
            tc.tile_pool(name="accpool", bufs=1) as accpool,
            tc.tile_pool(name="pspool", bufs=2, space="PSUM") as pspool,
        ):
            # constants
            ones_bf = accpool.tile([P, P], BF16, name="ones_bf")
            nc.vector.memset(ones_bf, 1.0)
            ones_f32 = accpool.tile([P, 1], F32, name="ones_f32")
            nc.vector.memset(ones_f32, 1.0)

            # persistent normalized operands (scaled by 1/sqrt(tau)), bf16
            a_sb = [abpool.tile([P, NCAT], BF16, name=f"a{k}") for k in range(KT)]
            b_sb = [abpool.tile([P, NCAT], BF16, name=f"b{k}") for k in range(KT)]
            cacc = accpool.tile([P, NF], F32, name="cacc")
            rs = {
                m: accpool.tile([P, 32], F32, name=f"rs{m}") for m in ("A", "B", "C")
            }

            # ---------------- normalization ----------------
            # For each side: load z [256, 6144] fp32 (2 partition tiles),
            # square, column-sum via all-ones matmul (result broadcast to all
            # 128 partitions), rnorm = sqrt((1/tau) * 1/colsum), then
            # a = z * rnorm  (bf16).
            for zdram, dst in ((zc1, a_sb), (zc2, b_sb)):
                zts = []
                sqs = []
                for k in range(KT):
                    zt = zpool.tile([P, NCAT], F32, tag="z", name=f"z{k}")
                    nc.sync.dma_start(out=zt, in_=zdram[k * P : (k + 1) * P, :])
                    sq = sqpool.tile([P, NCAT], BF16, tag="sq", name=f"sq{k}")
                    nc.vector.tensor_mul(sq, zt, zt)
                    zts.append(zt)
                    sqs.append(sq)
                for ch in range(NCAT // CH):  # 12 chunks
                    sl = slice(ch * CH, (ch + 1) * CH)
                    psn = pspool.tile([P, CH], F32, tag="ps", name="psn")
                    for k in range(KT):
                        nc.tensor.matmul(
                            psn,
                            ones_bf,
                            sqs[k][:, sl],
                            start=(k == 0),
                            stop=(k == KT - 1),
                        )
                    rbc = rnpool.tile([P, CH], F32, tag="rn", name="rbc")
                    nc.vector.reciprocal(rbc, psn)
                    rnb = rnpool.tile([P, CH], F32, tag="rn", name="rnb")
                    nc.scalar.activation(out=rnb, in_=rbc, func=Act.Sqrt, scale=1.0 / TAU)
                    for k in range(KT):
                        nc.vector.tensor_mul(dst[k][:, sl], zts[k][:, sl], rnb)

            # ---------------- main loop ----------------
            # 3 Gram products, rows = this core's 2048 (16 tiles of 128, lhsT
            # taken from concat columns [NF:]), cols = all 4096 (2 stripes of
            # 2048 = 4 psum banks each). exp + rowsum fused on ScalarE via
            # accum_out; C's exp tiles accumulated on VectorE for colsums.
            prods = (("A", a_sb, a_sb), ("B", b_sb, b_sb), ("C", a_sb, b_sb))
            for I in range(NH // P):  # 16
                lo = NF + I * P
                for pname, lt, rt in prods:
                    for h in range(NF // STRIPE):  # 2
                        ps = pspool.tile([P, STRIPE], F32, tag="ps", name="ps_mm")
                        for j4 in range(STRIPE // CH):  # 4
                            osl = slice(j4 * CH, (j4 + 1) * CH)
                            col = h * STRIPE + j4 * CH
                            for k in range(KT):
                                nc.tensor.matmul(
                                    ps[:, osl],
                                    lt[k][:, lo : lo + P],
                                    rt[k][:, col : col + CH],
                                    start=(k == 0),
                                    stop=(k == KT - 1),
                                )
                        if pname == "C":
                            e = ecpool.tile([P, STRIPE], BF16, tag="ec", name="ec")
                        else:
                            e = scrpool.tile([P, STRIPE], BF16, tag="scr", name="escr")
                        col_acc = rs[pname][:, I * 2 + h : I * 2 + h + 1]
                        nc.scalar.activation(
                            out=e, in_=ps, func=Act.Exp, accum_out=col_acc
                        )
                        if pname == "C":
                            csl = slice(h * STRIPE, (h + 1) * STRIPE)
                            if I == 0:
                                nc.vector.tensor_copy(cacc[:, csl], e)
                            else:
                                nc.vector.tensor_add(cacc[:, csl], cacc[:, csl], e)

            # ---------------- finalize ----------------
            # rowsums: rs[m] is [128, 16 I x 2 h] -> sum the h pairs.
            for m in ("A", "B", "C"):
                sf = accpool.tile([P, 16], F32, name=f"sfin{m}")
                nc.vector.tensor_reduce(
                    sf,
                    rs[m].rearrange("p (i h) -> p i h", h=2),
                    axis=mybir.AxisListType.X,
                    op=mybir.AluOpType.add,
                )
                off = {"A": 0, "B": NH, "C": 2 * NH}[m]
                nc.sync.dma_start(
                    out=out_t[off : off + NH].rearrange("(p i) -> p i", i=16),
                    in_=sf,
                )

            # colsum partials of exp(C): partition-reduce cacc via ones matmul
            for ch in range(NF // CH):  # 8
                psc = pspool.tile([1, CH], F32, tag="ps", name="psc")
                nc.tensor.matmul(
                    psc,
                    ones_f32,
                    cacc[:, ch * CH : (ch + 1) * CH],
                    start=True,
                    stop=True,
                )
                row = rnpool.tile([1, CH], F32, tag="row", name="csrow")
                nc.vector.tensor_copy(row, psc)
                o0 = 3 * NH + ch * CH
                nc.sync.dma_start(out=out_t[o0 : o0 + CH], in_=row)

            # dots: elementwise a*b over the lhsT columns, partition-reduce
            dms = []
            for k in range(KT):
                dm = scrpool.tile([P, STRIPE], BF16, tag="scr", name=f"dm{k}")
                nc.vector.tensor_mul(dm, a_sb[k][:, NF:], b_sb[k][:, NF:])
                dms.append(dm)
            for ch in range(NH // CH):  # 4
                psd = pspool.tile([1, CH], F32, tag="ps", name="psd")
                for k in range(KT):
                    nc.tensor.matmul(
                        psd,
                        ones_bf[:, 0:1],
                        dms[k][:, ch * CH : (ch + 1) * CH],
                        start=(k == 0),
                        stop=(k == KT - 1),
                    )
                row = rnpool.tile([1, CH], F32, tag="row", name="dotrow")
                nc.vector.tensor_copy(row, psd)
                o0 = 3 * NH + NF + ch * CH
                nc.sync.dma_start(out=out_t[o0 : o0 + CH], in_=row)

    nc.compile()
    return nc


def _get_program():
    global _PROGRAM
    if _PROGRAM is None:
        _PROGRAM = _build_program()
    return _PROGRAM


def _run_cores(z1, z2, **run_kwargs):
    """Shard, run the SPMD program on 8 cores, return per-core result dicts."""
    nc = _get_program()
    z1 = np.ascontiguousarray(np.asarray(z1, dtype=np.float32)).reshape(4, C, NF)
    z2 = np.ascontiguousarray(np.asarray(z2, dtype=np.float32)).reshape(4, C, NF)
    in_maps = []
    for core in range(8):
        b, half = core // 2, core % 2
        sl = slice(half * NH, (half + 1) * NH)
        in_maps.append(
            {
                "zc1": np.concatenate([z1[b], z1[b][:, sl]], axis=1),
                "zc2": np.concatenate([z2[b], z2[b][:, sl]], axis=1),
            }
        )
    res = run_bass_kernel_spmd(nc, in_maps, list(range(8)), **run_kwargs)
    return res


def _combine(results):
    """Host-side final math: tiny [4096]-vector ops + mean."""
    e0 = np.exp(1.0 / TAU)
    losses = []
    for b in range(4):
        parts = [np.asarray(results[2 * b + h]["out"], dtype=np.float64) for h in (0, 1)]
        sA, sB, sC, dots = [], [], [], []
        for p in parts:
            sA.append(p[0:NH].reshape(P, 16).T.reshape(-1))
            sB.append(p[NH : 2 * NH].reshape(P, 16).T.reshape(-1))
            sC.append(p[2 * NH : 3 * NH].reshape(P, 16).T.reshape(-1))
            dots.append(p[3 * NH + NF :])
        sA = np.concatenate(sA)
        sB = np.concatenate(sB)
        sC = np.concatenate(sC)
        dots = np.concatenate(dots)
        tC = parts[0][3 * NH : 3 * NH + NF] + parts[1][3 * NH : 3 * NH + NF]
        l1 = np.log(sA + sC - e0) - dots
        l2 = np.log(sB + tC - e0) - dots
        losses.append(0.5 * (l1 + l2))
    return np.array(np.mean(losses), dtype=np.float32)


def kernel(z1, z2):
    results = _run_cores(z1, z2).results
    return _combine(results)
